# revision 1
# baseline (speedup 1.0000x reference)
"""Causal MHA (B=4, T=2048, D=1024, H=16, Dh=64) on 8 TRN2 NeuronCores.

Sharding: tensor-parallel over heads (2 groups of 8 heads; W_q/W_k/W_v split
column-wise, W_o row-wise) x data-parallel over batch (4 batches). Core
c = (b, g) computes a partial output x[b] attention with head-group g; the
host sums the two head-group partials per batch.

Per-core kernel (Bass/Tile, fp32 data with float32r matmuls):
  A: load x [T,D], PE-transpose to x^T [D,T]
  B: Q^T = Wq^T x^T, K^T = Wk^T x^T (both [I,T]), V = x Wv (natural [T,I],
     stored per-head with a ones column for the softmax denominator)
  C: per head h, q-block of 512: S^T[k,q] = K_h^T(stationary) Q_h^T, exp on
     ACT (scale folded), causal handled per 128x512 tile (full tiles below
     the diagonal, lower-tri mask multiply on the diagonal square), then
     ctx^T[65,q] accumulates V_aug^T P^T in PSUM; row 64 is the denominator.
     Normalize via DVE reciprocal + ones-matmul partition broadcast.
  D: out = ctx^T.T Wo accumulated over inner chunks, DMA PSUM -> DRAM.
"""

import numpy as np

import concourse.bass as bass
import concourse.mybir as mybir
import concourse.tile as tile
from concourse import bacc
from concourse.bass_utils import run_bass_kernel_spmd
from concourse.masks import make_identity

B, T, D = 4, 2048, 1024
H_TOT, DH = 16, 64
N_CORES = 8
HPC = 8                  # heads per core
I = HPC * DH             # 512: inner width per core
F32 = mybir.dt.float32
F32R = mybir.dt.float32r
SCALE = float(DH) ** -0.5

_NC_CACHE = []


def _emit(nc, tc, ctx):
    from contextlib import ExitStack

    x_d = nc.dram_tensor("x", [T, D], F32, kind="ExternalInput")
    wq_d = nc.dram_tensor("wq", [D, I], F32, kind="ExternalInput")
    wk_d = nc.dram_tensor("wk", [D, I], F32, kind="ExternalInput")
    wv_d = nc.dram_tensor("wv", [D, I], F32, kind="ExternalInput")
    wo_d = nc.dram_tensor("wo", [I, D], F32, kind="ExternalInput")
    o_d = nc.dram_tensor("o", [T, D], F32, kind="ExternalOutput")

    xr = x_d.ap().bitcast(F32R).rearrange("(n p) d -> n p d", p=128)   # [16,128,1024]
    o_view = o_d.ap().rearrange("(n p) d -> n p d", p=128)             # [16,128,1024]

    persist = ctx.enter_context(tc.tile_pool(name="persist", bufs=1))
    psum_mm = ctx.enter_context(tc.tile_pool(name="psum_mm", bufs=3, space="PSUM"))
    psum_ctx = ctx.enter_context(tc.tile_pool(name="psum_ctx", bufs=2, space="PSUM"))
    psum_bc = ctx.enter_context(tc.tile_pool(name="psum_bc", bufs=2, space="PSUM"))

    # f32r cannot be memset directly; build constants in f32, then copy-round.
    ident32 = persist.tile([128, 128], F32, tag="ident32")
    make_identity(nc, ident32[:])
    ident = persist.tile([128, 128], F32R, tag="ident")
    nc.vector.tensor_copy(ident[:], ident32[:])
    # lower-triangular keep mask: ltri[i,j] = 1.0 if i <= j else 0.0
    ltri32 = persist.tile([128, 128], F32, tag="ltri32")
    nc.gpsimd.memset(ltri32[:], 1.0)
    nc.gpsimd.affine_select(
        out=ltri32[:], in_=ltri32[:], compare_op=mybir.AluOpType.is_ge,
        fill=0.0, base=0, pattern=[[1, 128]], channel_multiplier=-1,
    )
    ltri = persist.tile([128, 128], F32R, tag="ltri")
    nc.vector.tensor_copy(ltri[:], ltri32[:])
    ones64_32 = persist.tile([1, 64], F32, tag="ones64_32")
    nc.gpsimd.memset(ones64_32[:], 1.0)
    ones64 = persist.tile([1, 64], F32R, tag="ones64")
    nc.vector.tensor_copy(ones64[:], ones64_32[:])
    onescol32 = persist.tile([128, HPC, 1], F32, tag="onescol32")
    nc.gpsimd.memset(onescol32[:], 1.0)

    qT = [persist.tile([128, T], F32R, tag=f"qT{i}", name=f"qT{i}") for i in range(4)]
    kT = [persist.tile([128, T], F32R, tag=f"kT{i}", name=f"kT{i}") for i in range(4)]
    v3 = [persist.tile([128, HPC, DH + 1], F32R, tag=f"v{t}", name=f"v{t}") for t in range(16)]

    with ExitStack() as stage_ab:
        wpool = stage_ab.enter_context(tc.tile_pool(name="weights_qkv", bufs=1))
        xpool = stage_ab.enter_context(tc.tile_pool(name="x_nat", bufs=3))
        xTpool = stage_ab.enter_context(tc.tile_pool(name="xT", bufs=1))

        wq_t = wpool.tile([128, 8, I], F32R, tag="wq")
        wk_t = wpool.tile([128, 8, I], F32R, tag="wk")
        wv_t = wpool.tile([128, 8, I], F32R, tag="wv")
        for w_t, w_d in ((wq_t, wq_d), (wk_t, wk_d), (wv_t, wv_d)):
            nc.sync.dma_start(w_t[:], w_d.ap().bitcast(F32R).rearrange("(c p) i -> p c i", p=128))

        for th in range(2):  # halves of T, to bound SBUF usage
            xTs = [xTpool.tile([128, 1024], F32R, tag=f"xT{dc}", name=f"xT_{th}_{dc}") for dc in range(8)]
            # stage A: transpose this half of x
            for tt in range(8):
                gt = th * 8 + tt
                x_t = xpool.tile([128, D], F32R, tag="x_t")
                nc.sync.dma_start(x_t[:], xr[gt])
                for dc in range(8):
                    tp = psum_mm.tile([128, 128], F32R, tag="mm")
                    nc.tensor.transpose(tp[:], x_t[:, dc * 128:(dc + 1) * 128], ident[:])
                    nc.any.tensor_copy(xTs[dc][:, tt * 128:(tt + 1) * 128], tp[:])
            # stage B: Q^T, K^T (transposed layouts)
            for ic in range(4):
                for tb in range(2):
                    t0 = th * 1024 + tb * 512
                    for w_t, dstT in ((wq_t, qT), (wk_t, kT)):
                        ps = psum_mm.tile([128, 512], F32, tag="mm")
                        for dc in range(8):
                            nc.tensor.matmul(
                                ps[:],
                                w_t[:, dc, ic * 128:(ic + 1) * 128],
                                xTs[dc][:, tb * 512:(tb + 1) * 512],
                                start=(dc == 0), stop=(dc == 7),
                            )
                        nc.any.tensor_copy(dstT[ic][:, t0:t0 + 512], ps[:])
            # stage B: V natural, per-head columns + ones column
            for tt in range(8):
                gt = th * 8 + tt
                ps = psum_mm.tile([128, 512], F32, tag="mm")
                for dc in range(8):
                    nc.tensor.matmul(
                        ps[:],
                        xTs[dc][:, tt * 128:(tt + 1) * 128],
                        wv_t[:, dc, :],
                        start=(dc == 0), stop=(dc == 7),
                    )
                nc.any.tensor_copy(
                    v3[gt][:, :, 0:DH],
                    ps[:].rearrange("p (h d) -> p h d", h=HPC),
                )
                nc.vector.tensor_copy(v3[gt][:, :, DH:DH + 1], onescol32[:])

    # stage C: attention
    ctxTpool = ctx.enter_context(tc.tile_pool(name="ctxT", bufs=1))
    ctxT = [ctxTpool.tile([128, T], F32R, tag=f"ctxT{i}", name=f"ctxT{i}") for i in range(4)]
    ptpool = ctx.enter_context(tc.tile_pool(name="pt", bufs=4))
    recpool = ctx.enter_context(tc.tile_pool(name="rec", bufs=2))
    bcspool = ctx.enter_context(tc.tile_pool(name="bcs", bufs=2))
    wopool = ctx.enter_context(tc.tile_pool(name="wo", bufs=1))
    wo_t = wopool.tile([128, 4, D], F32R, tag="wo")
    nc.sync.dma_start(wo_t[:], wo_d.ap().bitcast(F32R).rearrange("(c p) d -> p c d", p=128))

    for h in range(HPC):
        ti, po = h // 2, (h % 2) * 64
        for qb in range(4):
            q0 = qb * 512
            n_kt = 4 * (qb + 1)
            cps = psum_ctx.tile([DH + 1, 512], F32, tag="ctx")
            for kt in range(n_kt):
                k0 = kt * 128
                m = kt - 4 * qb  # >= 0 means this k-tile touches the diagonal
                # Diagonal tiles only contribute to q columns >= c0; columns
                # below c0 belong entirely to earlier k-tiles, so slice them
                # out of scores/exp/PV instead of masking them to zero.
                c0 = max(m, 0) * 128
                sps = psum_mm.tile([128, 512], F32, tag="mm")
                nc.tensor.matmul(
                    sps[:, c0:512],
                    kT[ti][po:po + 64, k0:k0 + 128],
                    qT[ti][po:po + 64, q0 + c0:q0 + 512],
                    start=True, stop=True,
                )
                pt = ptpool.tile([128, 512], F32R, tag="pt")
                nc.scalar.activation(
                    pt[:, c0:512], sps[:, c0:512],
                    mybir.ActivationFunctionType.Exp, scale=SCALE,
                )
                if m >= 0:
                    nc.vector.tensor_mul(pt[:, c0:c0 + 128], pt[:, c0:c0 + 128], ltri[:])
                nc.tensor.matmul(
                    cps[:, c0:512], v3[kt][:, h, :], pt[:, c0:512],
                    start=(kt == 0), stop=(kt == n_kt - 1),
                )
            rec = recpool.tile([1, 512], F32R, tag="rec")
            with nc.allow_low_precision(reason="softmax denom reciprocal rounded to f32r; feeds f32r broadcast matmul"):
                nc.vector.reciprocal(rec[:], cps[DH:DH + 1, :])
            bcp = psum_bc.tile([64, 512], F32, tag="bc")
            nc.tensor.matmul(bcp[:], ones64[:], rec[:], start=True, stop=True)
            bcs = bcspool.tile([64, 512], F32, tag="bcs")
            nc.any.tensor_copy(bcs[:], bcp[:])
            nc.vector.tensor_mul(ctxT[ti][po:po + 64, q0:q0 + 512], cps[0:DH, :], bcs[:])

    # stage D: output projection, PSUM -> SBUF -> DRAM
    outpool = ctx.enter_context(tc.tile_pool(name="out_sb", bufs=3))
    for tt in range(16):
        for db in range(2):
            ops = psum_mm.tile([128, 512], F32, tag="mm")
            for ic in range(4):
                nc.tensor.matmul(
                    ops[:],
                    ctxT[ic][:, tt * 128:(tt + 1) * 128],
                    wo_t[:, ic, db * 512:(db + 1) * 512],
                    start=(ic == 0), stop=(ic == 3),
                )
            osb = outpool.tile([128, 512], F32, tag="osb")
            nc.any.tensor_copy(osb[:], ops[:])
            nc.sync.dma_start(o_view[tt][:, db * 512:(db + 1) * 512], osb[:])


def _build():
    from contextlib import ExitStack

    nc = bacc.Bacc("TRN2", target_bir_lowering=False, debug=False,
                   enable_asserts=True, num_devices=N_CORES)
    with tile.TileContext(nc) as tc:
        with ExitStack() as ctx:
            _emit(nc, tc, ctx)
    nc.compile()
    return nc


def _get_nc():
    if not _NC_CACHE:
        _NC_CACHE.append(_build())
    return _NC_CACHE[0]


def _in_maps(x, W_q, W_k, W_v, W_o):
    maps = []
    for c in range(N_CORES):
        b, g = c // 2, c % 2
        s = slice(g * I, (g + 1) * I)
        maps.append({
            "x": np.ascontiguousarray(x[b]),
            "wq": np.ascontiguousarray(W_q[:, s]),
            "wk": np.ascontiguousarray(W_k[:, s]),
            "wv": np.ascontiguousarray(W_v[:, s]),
            "wo": np.ascontiguousarray(W_o[s, :]),
        })
    return maps


def kernel(**inputs):
    x = np.asarray(inputs["x"], dtype=np.float32)
    W_q = np.asarray(inputs["W_q"], dtype=np.float32)
    W_k = np.asarray(inputs["W_k"], dtype=np.float32)
    W_v = np.asarray(inputs["W_v"], dtype=np.float32)
    W_o = np.asarray(inputs["W_o"], dtype=np.float32)

    nc = _get_nc()
    res = run_bass_kernel_spmd(nc, _in_maps(x, W_q, W_k, W_v, W_o),
                               core_ids=list(range(N_CORES)))
    out = np.empty((B, T, D), dtype=np.float32)
    for b in range(B):
        out[b] = res.results[2 * b]["o"] + res.results[2 * b + 1]["o"]
    return out

